# revision 1
# baseline (speedup 1.0000x reference)
"""Trainium2 Bass kernel for nn_Network_61658550501610 (Mamba block + MLP head).

Reference computation (per batch element b, sequence length L=2048):
  xz = x @ W_in.T; xi, z = split(xz)
  xc = silu(causal_depthwise_conv(xi, conv_w) + conv_b)
  x_dbl = xc @ W_xproj.T -> (dt, B, C)
  delta = softplus(dt @ W_dt.T + b_dt)
  h_t = exp(delta*A)*h_{t-1} + delta*B*xc   (selective scan, state [82,16])
  y = (h @ C) + D*xc; y *= silu(z)
  out = y @ W_out.T;  logits = relu(out@W_c1.T+b_c1)@W_c2.T + b_c2

Sharding: data-parallel over batch (B=16 -> 2 per core across 8 cores).

Layout: d_inner (82) on partitions, time on free dim; x is pre-transposed,
left-padded by K-1 and augmented with a ones row on host, so the depthwise
conv + input projection + conv bias fold into 4 shifted accumulating
matmuls.  The scan packs rows r=(n,dsub) -> 128 partitions x 11 d-groups;
delta/u are broadcast to that layout with TensorE selection matmuls (the
delta selector is pre-scaled by A so dA = exp() needs no per-partition
scale and group pairs share one 1024-col exp).  B/C broadcasts are folded
into the x_proj weights.  dBx is one stride-0-broadcast DVE multiply over
all 11 groups, and the 11 per-group scans collapse into ONE scan
instruction per chunk by zeroing dA's first column per group and folding
the group carry into dBx's first column.  The D*xc skip term runs as an
accumulating diagonal matmul into y.
"""
import ml_dtypes
import numpy as np

import concourse.bacc as bacc
import concourse.tile as tile
import concourse.mybir as mybir
from concourse.bass_utils import run_bass_kernel_spmd

F32 = mybir.dt.float32
F32R = mybir.dt.float32r
BF16 = mybir.dt.bfloat16
OP = mybir.AluOpType
ACTF = mybir.ActivationFunctionType

# problem dims (hardcoded per contract)
B, L, DM = 16, 2048, 41
DIN, N, K = 82, 16, 4          # d_inner, d_state, d_conv
DTR, HID, NL = 3, 64, 10
NCORES = 8
BLOC = B // NCORES             # batch per core

DM1 = DM + 1                   # + ones row (folds conv_b)
DG = (DIN + 7) // 8            # 11 d-groups of 8 for the packed scan
DP = DG * 8                    # 88 padded d
C = 512                        # time-chunk length
NCH = L // C                   # chunks per batch element
Q = C // 128                   # 128-row subtiles per chunk

_cache = {}


def _build(cfg):
    nc = bacc.Bacc("TRN2", target_bir_lowering=False, debug=False,
                   enable_asserts=False)

    def din(name, shape):
        return nc.dram_tensor(name, list(shape), F32, kind="ExternalInput").ap()

    xT_d = nc.dram_tensor("xT", [BLOC, DM1, L + K - 1], F32R,
                          kind="ExternalInput").ap()
    w_zT_d = nc.dram_tensor("w_zT", [DM1, DIN], F32R,
                            kind="ExternalInput").ap()
    w_cvT_d = nc.dram_tensor("w_cvT", [DM1, K * DIN], F32R,
                             kind="ExternalInput").ap()
    w_effT_d = nc.dram_tensor("w_effT", [DIN, DIN], F32R,
                              kind="ExternalInput").ap()
    b_dt_d = din("b_dt", (DIN, 1))
    d_diag_d = nc.dram_tensor("d_diag", [DIN, DIN], F32R,
                              kind="ExternalInput").ap()
    w_bq_d = nc.dram_tensor("w_bq", [DIN, 128], F32R,
                            kind="ExternalInput").ap()
    w_cq_d = nc.dram_tensor("w_cq", [DIN, 128], F32R,
                            kind="ExternalInput").ap()
    w1T_d = nc.dram_tensor("w1T", [DIN, HID], F32R,
                           kind="ExternalInput").ap()
    b_c1_d = din("b_c1", (HID, 1))
    w2T_d = nc.dram_tensor("w2T", [HID + 1, NL], BF16,
                           kind="ExternalInput").ap()
    p_sela_d = nc.dram_tensor("p_sela", [DIN, DG * 128], BF16,
                              kind="ExternalInput").ap()
    p_sel1_d = nc.dram_tensor("p_sel1", [DIN, DG * 128], BF16,
                              kind="ExternalInput").ap()
    ed_sel_d = nc.dram_tensor("ed_sel", [128, DG * DP], BF16,
                              kind="ExternalInput").ap()
    out_d = nc.dram_tensor("out", [BLOC, L, NL], F32, kind="ExternalOutput").ap()

    with tile.TileContext(nc) as tc, tc.tile_pool(name="wts", bufs=1) as wp, \
         tc.tile_pool(name="work", bufs=4) as kp, \
         tc.tile_pool(name="da", bufs=2) as dap, \
         tc.tile_pool(name="ua", bufs=2) as uap, \
         tc.tile_pool(name="dbx", bufs=2) as dbp, \
         tc.tile_pool(name="hc", bufs=1) as hcp, \
         tc.tile_pool(name="hbuf", bufs=2) as hp, \
         tc.tile_pool(name="ps_f", bufs=2, space="PSUM") as pf, \
         tc.tile_pool(name="ps_rep", bufs=2, space="PSUM") as prep, \
         tc.tile_pool(name="ps_y", bufs=2, space="PSUM") as py:

        # ---- constant weights ----
        w_zT = wp.tile([DM1, DIN], F32R)
        w_cvT = wp.tile([DM1, K * DIN], F32R)
        w_effT = wp.tile([DIN, DIN], F32R)
        b_dt = wp.tile([DIN, 1], F32)
        d_diag = wp.tile([DIN, DIN], F32R)
        w_bq = wp.tile([DIN, 128], F32R)
        w_cq = wp.tile([DIN, 128], F32R)
        w1T = wp.tile([DIN, HID], F32R)
        b_c1 = wp.tile([HID, 1], F32)
        w2T = wp.tile([HID + 1, NL], BF16)
        p_sela = wp.tile([DIN, DG * 128], BF16)
        p_sel1 = wp.tile([DIN, DG * 128], BF16)
        ed_sel = wp.tile([128, DG * DP], BF16)
        for t_, d_ in [(w_zT, w_zT_d), (w_cvT, w_cvT_d), (w_effT, w_effT_d),
                       (b_dt, b_dt_d), (d_diag, d_diag_d),
                       (w_bq, w_bq_d), (w_cq, w_cq_d), (w1T, w1T_d),
                       (b_c1, b_c1_d), (w2T, w2T_d),
                       (p_sela, p_sela_d), (p_sel1, p_sel1_d),
                       (ed_sel, ed_sel_d)]:
            nc.sync.dma_start(t_[:], d_[:])

        # persistent state, one per batch element (independent streams)
        h_carry_b = [wp.tile([128, DG], F32, name=f"hcar{i}", tag=f"hcar{i}")
                     for i in range(BLOC)]
        for t_ in h_carry_b:
            nc.vector.memset(t_[:], 0.0)
        # gating-head scratch with a persistent all-ones bias row
        g_aug_p = [wp.tile([HID + 1, C], BF16, name=f"gaug{i}", tag=f"gaug{i}")
                   for i in range(2)]
        for t_ in g_aug_p:
            nc.vector.memset(t_[HID:HID + 1, :], 1.0)

        def front(ch, b):
            t0 = ch * C
            # ---- load x chunk [DM+1, C+3] (pre-transposed, padded, ones) --
            xT = kp.tile([DM1, C + K - 1], F32R, tag="xT", bufs=3)
            nc.sync.dma_start(xT[:], xT_d[b, :, t0:t0 + C + K - 1])

            # ---- z and conv(xi)+conv_b (ones row carries the bias) ----
            z_ps = pf.tile([DIN, C], F32, tag="f")
            nc.tensor.matmul(z_ps[:], w_zT[:],
                             xT[:, K - 1:K - 1 + C], start=True, stop=True)
            xcp_ps = pf.tile([DIN, C], F32, tag="f")
            for k in range(K):
                nc.tensor.matmul(xcp_ps[:],
                                 w_cvT[:, k * DIN:(k + 1) * DIN],
                                 xT[:, k:k + C], start=(k == 0),
                                 stop=(k == K - 1))
            # silu on both halves: s = 0.5*tanh(v/2)+0.5 ; out = v*s
            th_zc = kp.tile([DIN, 2 * C], BF16, tag="th_zc", bufs=3)
            nc.scalar.activation(th_zc[:, 0:C], z_ps[:], ACTF.Tanh,
                                 scale=0.5)
            nc.scalar.activation(th_zc[:, C:2 * C], xcp_ps[:], ACTF.Tanh,
                                 scale=0.5)
            sg_zc = kp.tile([DIN, 2 * C], F32, tag="sg_zc", bufs=3)
            nc.scalar.activation(sg_zc[:], th_zc[:], ACTF.Copy,
                                 bias=0.5, scale=0.5)
            zs = kp.tile([DIN, C], F32, tag="zs", bufs=4)
            nc.vector.tensor_tensor(zs[:], z_ps[:],
                                    sg_zc[:, 0:C], op=OP.mult)
            xc = kp.tile([DIN, C], F32, tag="xc", bufs=4)
            nc.vector.tensor_tensor(xc[:].bitcast(F32R), xcp_ps[:],
                                    sg_zc[:, C:2 * C], op=OP.mult)

            # ---- x_proj: delta, and B/C broadcast straight to 128 rows ----
            dpre_ps = pf.tile([DIN, C], F32, tag="f")
            nc.tensor.matmul(dpre_ps[:], w_effT[:],
                             xc[:].bitcast(F32R), start=True, stop=True)
            # softplus(v) = ln(exp(v) + 1), v = dpre + b_dt
            e_sp = kp.tile([DIN, C], F32, tag="e_sp", bufs=2)
            nc.scalar.activation(e_sp[:], dpre_ps[:], ACTF.Exp, bias=b_dt[:])
            delta = kp.tile([DIN, C], BF16, tag="delta", bufs=3)
            nc.scalar.activation(delta[:], e_sp[:], ACTF.Ln, bias=1.0)

            bq_ps = pf.tile([128, C], F32, tag="f")
            nc.tensor.matmul(bq_ps[:], w_bq[:],
                             xc[:].bitcast(F32R), start=True, stop=True)
            cq_ps = pf.tile([128, C], F32, tag="f")
            nc.tensor.matmul(cq_ps[:], w_cq[:],
                             xc[:].bitcast(F32R), start=True, stop=True)
            bc_sb = kp.tile([128, 2 * C], BF16, tag="bc_sb", bufs=4)
            nc.scalar.copy(bc_sb[:, 0:C], bq_ps[:])
            nc.scalar.copy(bc_sb[:, C:2 * C], cq_ps[:])

            # u = delta * xc (bf16, feeds the p_sel replication matmul)
            u = kp.tile([DIN, C], BF16, tag="u", bufs=3)
            nc.vector.tensor_tensor(u[:], delta[:], xc[:], op=OP.mult)

            return dict(delta=delta, u=u, bc_sb=bc_sb, xc=xc, zs=zs)

        def mid(j, ch, b, st):
            delta, u, bc_sb = st["delta"], st["u"], st["bc_sb"]
            # ---- replicate delta (A-scaled) and u to the packed layout,
            #      two groups per PSUM tile so evacuations run at 1024 cols
            dA_all = dap.tile([128, DG * C], BF16, tag="dA")
            u_all = uap.tile([128, DG * C], BF16, tag="uA")
            g = 0
            while g < DG:
                w = 2 if g + 1 < DG else 1
                dd_ps = prep.tile([128, 2 * C], F32, tag="rep")
                for i in range(w):
                    nc.tensor.matmul(dd_ps[:, i * C:(i + 1) * C],
                                     p_sela[:, (g + i) * 128:(g + i + 1) * 128],
                                     delta[:], start=True, stop=True)
                nc.scalar.activation(dA_all[:, g * C:(g + w) * C],
                                     dd_ps[:, 0:w * C], ACTF.Exp)
                uu_ps = prep.tile([128, 2 * C], F32, tag="rep")
                for i in range(w):
                    nc.tensor.matmul(uu_ps[:, i * C:(i + 1) * C],
                                     p_sel1[:, (g + i) * 128:(g + i + 1) * 128],
                                     u[:], start=True, stop=True)
                nc.scalar.copy(u_all[:, g * C:(g + w) * C], uu_ps[:, 0:w * C])
                g += w

            # ---- dBx for all groups in one multiply (b broadcast over g) ----
            dBx_all = dbp.tile([128, DG * C], BF16, tag="dbx")
            nc.vector.tensor_tensor(
                dBx_all[:].rearrange("p (g c) -> p g c", g=DG),
                u_all[:].rearrange("p (g c) -> p g c", g=DG),
                bc_sb[:, 0:C].unsqueeze(1).to_broadcast((128, DG, C)),
                op=OP.mult)

            st["dA_all"] = dA_all
            st["dBx_all"] = dBx_all

        def tail(j, ch, b, st):
            h_carry = h_carry_b[b]
            t0 = ch * C
            bc_sb, xc, zs = st["bc_sb"], st["xc"], st["zs"]
            dA_all, dBx_all = st["dA_all"], st["dBx_all"]

            # ---- scans; hC = h * C and the y accumulation start on the
            #      first half of the groups while the rest still scan, so
            #      TensorE overlaps the scan window
            h = hp.tile([128, DG * C], BF16, tag="h")
            hC = hcp.tile([128, DG * C], BF16, tag="hC")
            y_ps = py.tile([DP, C], F32, tag="y")
            GS = 6
            for g0, g1 in ((0, GS), (GS, DG)):
                for g in range(g0, g1):
                    init = 0.0 if ch == 0 else h_carry[:, g:g + 1]
                    nc.vector.tensor_tensor_scan(
                        h[:, g * C:(g + 1) * C], dA_all[:, g * C:(g + 1) * C],
                        dBx_all[:, g * C:(g + 1) * C], init,
                        op0=OP.mult, op1=OP.add)
                ng = g1 - g0
                nc.vector.tensor_tensor(
                    hC[:, g0 * C:g1 * C].rearrange("p (g c) -> p g c", g=ng),
                    h[:, g0 * C:g1 * C].rearrange("p (g c) -> p g c", g=ng),
                    bc_sb[:, C:2 * C].unsqueeze(1).to_broadcast((128, ng, C)),
                    op=OP.mult)
                for g in range(g0, g1):
                    nc.tensor.matmul(y_ps[:], ed_sel[:, g * DP:(g + 1) * DP],
                                     hC[:, g * C:(g + 1) * C],
                                     start=(g == 0), stop=False)
            if ch < NCH - 1:
                nc.vector.tensor_copy(
                    h_carry[:].rearrange("p (g c) -> p g c", c=1),
                    h[:].rearrange("p (g c) -> p g c", g=DG)[:, :, C - 1:C])
            # skip term D*xc as an accumulating diagonal matmul
            nc.tensor.matmul(y_ps[0:DIN, :], d_diag[:],
                             xc[:].bitcast(F32R), start=False, stop=True)

            # ---- gate + output head ----
            y_gated = kp.tile([DIN, C], F32, tag="y_g", bufs=2)
            nc.vector.tensor_tensor(y_gated[:].bitcast(F32R), y_ps[0:DIN, :],
                                    zs[:], op=OP.mult)

            g_ps = pf.tile([HID, C], F32, tag="f")
            nc.tensor.matmul(g_ps[:], w1T[:],
                             y_gated[:].bitcast(F32R), start=True, stop=True)
            g_aug = g_aug_p[j % 2]
            nc.scalar.activation(g_aug[0:HID, :], g_ps[:], ACTF.Relu,
                                 bias=b_c1[:])

            lg_ps = pf.tile([128, Q * NL], F32, tag="f")
            for q in range(Q):
                nc.tensor.matmul(lg_ps[:, q * NL:(q + 1) * NL],
                                 g_aug[:, q * 128:(q + 1) * 128],
                                 w2T[:], start=True, stop=True)
            out_sb = kp.tile([128, Q * NL], F32, tag="out_sb", bufs=2)
            nc.scalar.copy(out_sb[:], lg_ps[:])
            dst = out_d[b, t0:t0 + C, :].rearrange("(q p) c -> p q c", p=128)
            nc.sync.dma_start(
                dst, out_sb[:].rearrange("p (q c) -> p q c", q=Q))

        # 3-stage skewed software pipeline:
        #   step j issues front(j), mid(j-1), tail(j-2) so the scan of one
        #   chunk overlaps the replication of the next and the front of the
        #   one after.
        iters = [(ch, b) for ch in range(NCH) for b in range(BLOC)]
        nj = len(iters)
        sts = [None] * nj
        for j in range(nj + 2):
            if j < nj:
                ch, b = iters[j]
                sts[j] = (j, ch, b, front(ch, b))
            if 0 <= j - 1 < nj:
                mid(*sts[j - 1])
            if j - 2 >= 0:
                tail(*sts[j - 2])
                sts[j - 2] = None

    nc.compile()
    return nc


def _packed_consts(A):
    p_sela = np.zeros((DIN, DG * 128), np.float32)
    p_sel1 = np.zeros((DIN, DG * 128), np.float32)
    ed = np.zeros((128, DG * DP), np.float32)
    for n in range(N):
        for ds in range(8):
            r = n * 8 + ds
            for g in range(DG):
                d = g * 8 + ds
                if d < DIN:
                    p_sela[d, g * 128 + r] = A[d, n]
                    p_sel1[d, g * 128 + r] = 1.0
                    ed[r, g * DP + d] = 1.0
    bf = ml_dtypes.bfloat16
    return {"p_sela": p_sela.astype(bf), "p_sel1": p_sel1.astype(bf),
            "ed_sel": ed.astype(bf)}


def _prep_inputs(inputs):
    x = np.asarray(inputs["x"], np.float32)
    W_in = np.asarray(inputs["W_in"], np.float64)
    conv_w = np.asarray(inputs["conv_w"], np.float64)
    conv_b = np.asarray(inputs["conv_b"], np.float64)
    W_xproj = np.asarray(inputs["W_xproj"], np.float64)
    W_dt = np.asarray(inputs["W_dt"], np.float64)
    b_dt = np.asarray(inputs["b_dt"], np.float64)
    A_log = np.asarray(inputs["A_log"], np.float64)
    D = np.asarray(inputs["D"], np.float64)
    W_out = np.asarray(inputs["W_out"], np.float64)
    W_c1 = np.asarray(inputs["W_c1"], np.float64)
    b_c1 = np.asarray(inputs["b_c1"], np.float64)
    W_c2 = np.asarray(inputs["W_c2"], np.float64)
    b_c2 = np.asarray(inputs["b_c2"], np.float64)

    f = lambda a: np.ascontiguousarray(a, dtype=np.float32)
    W_in_xi, W_in_z = W_in[:DIN], W_in[DIN:]
    # fused conv+in_proj weights, ones row carries conv_b on tap 0
    w_cvT = np.zeros((DM1, K * DIN), np.float64)
    for k in range(K):
        w_cvT[:DM, k * DIN:(k + 1) * DIN] = (conv_w[:, k:k + 1] * W_in_xi).T
    w_cvT[DM, 0:DIN] = conv_b
    w_zT = np.zeros((DM1, DIN), np.float64)
    w_zT[:DM] = W_in_z.T

    w_bcT = W_xproj[DTR:].T                       # [82, 32]
    nmap = [r // 8 for r in range(128)]
    bf = ml_dtypes.bfloat16
    shared = {
        "w_zT": f(w_zT),
        "w_cvT": f(w_cvT),
        "w_effT": f((W_dt @ W_xproj[:DTR]).T),
        "b_dt": f(b_dt[:, None]),
        "d_diag": f(np.diag(D)),
        "w_bq": f(w_bcT[:, nmap]),
        "w_cq": f(w_bcT[:, [N + n for n in nmap]]),
        "w1T": f((W_c1 @ W_out).T),
        "b_c1": f(b_c1[:, None]),
        "w2T": np.vstack([W_c2.T, b_c2[None, :]]).astype(bf),
        **_packed_consts(-np.exp(A_log)),
    }
    in_maps = []
    for c in range(NCORES):
        m = dict(shared)
        xb = x[c * BLOC:(c + 1) * BLOC]           # [BLOC, L, DM]
        xt = np.zeros((BLOC, DM1, L + K - 1), np.float32)
        xt[:, :DM, K - 1:] = xb.transpose(0, 2, 1)
        xt[:, DM, :] = 1.0
        m["xT"] = xt
        in_maps.append(m)
    return in_maps


def kernel(**inputs):
    return _run(inputs, trace=False)[0]


def kernel_traced(**inputs):
    return _run(inputs, trace=True)


def _run(inputs, trace=False):
    key = "nc"
    if key not in _cache:
        _cache[key] = _build({})
    nc = _cache[key]
    in_maps = _prep_inputs(inputs)
    res = run_bass_kernel_spmd(nc, in_maps, core_ids=list(range(NCORES)),
                               trace=trace)
    out = np.concatenate([r["out"] for r in res.results], axis=0)
    return out, res

